# revision 16
# baseline (speedup 1.0000x reference)
"""GQA attention kernel (RoPE + causal softmax + out-proj) for 8 trn2 cores.

Sharding: core = b*4 + g (b = batch 0..1, g = kv-head group 0..3); each
core computes q-heads 4g..4g+3, kv head g, batch b; attention outputs
are AllGathered within each batch group (3 collectives) and each core
produces a distinct 512-column slice of the output projection.

Changes vs the 200us baseline (same-window A/B ratio ~0.77):
  - x host layout [p, n, k, c]: per 512-token block one contiguous 2MB
    descriptor; projections run n-outer so PE chases one block at a
    time.  In the timing loop, block 0 is software-pipelined across
    iterations (prefetched from inside qproj(3)).
  - RoPE restructured from 6 Pool ops to 3 DVE muls + 2 add/subs
    (cos/sin tables are duplicated-half so one [128,512] mul computes
    both real*c and imag*c).  Pool was the hidden critical path of the
    proj and attention phases (6.7us/call, 20 calls); rope is now
    ~1.65us/call on DVE which has slack in every phase.
  - attention-output AllGather payload in fp8 e3m4 (scale-by-4 folded
    into wv on the host, unscale folded into wo): halves the collective
    staging + gathered-tile reload traffic.  Measured rel err 1.45e-2
    vs the 2e-2 gate (error is dominated by this choice; drop ao8=True
    to return to ~5.6e-4 at ~+4% time).
  - out-projection reads the gathered fp8e3 tiles directly as the
    matmul stationary operand (mixed e3m4 x f16 runs at full PE rate).
"""

import sys

sys.path.insert(0, "/opt/trn_rl_repo")

import numpy as np

import concourse.bacc as bacc
import concourse.mybir as mybir
from concourse.tile import TileContext
from concourse.bass_utils import run_bass_kernel_spmd

B, T, DIM = 2, 2048, 2048
NH, KVH, HD = 16, 4, 128
GQ = NH // KVH          # q heads per core = 4
KT = DIM // 128         # 16 contraction tiles
NT = T // 512           # 4 tq tiles of 512
F16 = mybir.dt.float16
F32 = mybir.dt.float32
EXP = mybir.ActivationFunctionType.Exp
EXP_BIAS = -5.0
MAP01 = [0, 1, 4, 5, 8, 9, 12, 13]   # ao_c01 row-block -> global head
MAP2 = [2, 6, 10, 14]
MAP3 = [3, 7, 11, 15]


def build_nc(reps=1, phase="all", fake_ag=False, skip_loads=False,
             dup_loads=False, chunk=0, one_ring=False, ao8=True,
             look=2, ar_dve=True, pa_fill=False, ps33=True):
    nc = bacc.Bacc("TRN2", target_bir_lowering=False, debug=False,
                   num_devices=8)
    # x: [128, NT, KT*512] -- element [p, n, 512k+c] = x[128k+p, 512n+c]
    xT = nc.dram_tensor("xT", [128, NT, KT * 512], F16, kind="ExternalInput")
    wq = nc.dram_tensor("wq", [128, KT, 512], F16, kind="ExternalInput")
    wkv = nc.dram_tensor("wkv", [128, 4 * 2048], F16,
                         kind="ExternalInput")
    wo = nc.dram_tensor("wo", [128, KT, 512], F16, kind="ExternalInput")
    masks = nc.dram_tensor("masks", [128, 2048], F16, kind="ExternalInput")
    ident = nc.dram_tensor("ident", [128, 128], F16, kind="ExternalInput")
    y = nc.dram_tensor("y", [T, 512], F16, kind="ExternalOutput")
    tens = dict(xT=xT, wq=wq, wkv=wkv, wo=wo,
                masks=masks, ident=ident, y=y)

    with TileContext(nc) as tc:
        with (
            tc.tile_pool(name="sb", bufs=1) as sb,
            tc.tile_pool(name="ps", bufs=1, space="PSUM") as ps,
            tc.tile_pool(name="dram", bufs=1, space="DRAM") as dram,
        ):
            if reps == 1:
                _body(nc, tc, sb, ps, dram, tens, fake_ag=fake_ag,
                      phase=phase, loop=False, skip_loads=skip_loads,
                      dup_loads=dup_loads, chunk=chunk, one_ring=one_ring,
                      ao8=ao8, look=look, ar_dve=ar_dve, pa_fill=pa_fill,
                      ps33=ps33)
            else:
                with tc.For_i(0, reps, 1):
                    _body(nc, tc, sb, ps, dram, tens, fake_ag=fake_ag,
                          phase=phase, loop=True, skip_loads=skip_loads,
                          dup_loads=dup_loads, chunk=chunk, one_ring=one_ring,
                      ao8=ao8, look=look, ar_dve=ar_dve, pa_fill=pa_fill,
                      ps33=ps33)
    nc.compile()
    return nc


def _body(nc, tc, sb, ps, dram, tens, fake_ag=False, phase="all",
          loop=False, skip_loads=False, dup_loads=False, chunk=0,
          one_ring=False, ao8=False, look=3, ar_dve=False,
          pa_fill=False, ps33=False):
    SBLK_B = 3 if ps33 else 4
    PROJ_B = 3 if ps33 else 2
    F8 = mybir.dt.float8e3
    AOD = F8 if ao8 else F16
    def ld(eng, dst, src):
        if skip_loads:
            return
        if one_ring:
            eng = nc.sync
        dsh, ssh = list(dst.shape), list(src.shape)
        n = dsh[-1]
        if chunk and n > chunk:
            # split along dst's last axis into <=chunk-col pieces
            for c0 in range(0, n, chunk):
                c1 = min(n, c0 + chunk)
                if len(ssh) == 2:
                    eng.dma_start(dst[:, c0:c1], src[:, c0:c1])
                else:
                    # src [128, K, C]; dst col = k*C + c
                    C = ssh[2]
                    assert c0 % C == 0 and c1 % C == 0, (c0, c1, C)
                    eng.dma_start(dst[:, c0:c1], src[:, c0 // C:c1 // C, :])
            return
        eng.dma_start(dst, src)
        if dup_loads:
            eng.dma_start(dst, src)
    if phase == "empty":
        osbE = sb.tile([128, 512], F16, tag="osb", bufs=2, name="osbE")
        inE = sb.tile([128, 512], F16, tag="inE", bufs=2, name="inE")
        nc.sync.dma_start(inE[:], tens["xT"][:, 0, 0:512])
        nc.vector.tensor_copy(osbE[:], inE[:])
        nc.sync.dma_start(tens["y"][0:128, :], osbE[:])
        return

    # ---- resident tiles ---------------------------------------------
    # x blocks: 4 separate bufs=1 tags so each slot's cross-iteration WAR
    # is exactly "prev iteration's readers of THIS block" (done by
    # mid-attention-2 via qproj3), letting reloads start at the boundary.
    xb = [sb.tile([128, KT * 512], F16, tag=f"xb{n}", bufs=1,
                  name=f"xb{n}") for n in range(NT)]
    xcol = [[xb[n][:, 512 * k:512 * (k + 1)] for k in range(KT)]
            for n in range(NT)]

    # scalar ring order: wk, wv, ck, sk, wqA(4 k-tiles), [xb3, wqB, wo
    # emitted later in program order]
    # wk|wv|ck|sk merged into one 2MB blob: one DMA, and in the loop
    # build it is prefetched for the NEXT iteration from inside qproj(2)
    # (all readers -- k/v-proj and the ropes -- are done by attention 1),
    # so the first k-proj matmul of an iteration never waits on DMA.
    wkv_sb = sb.tile([128, 4 * 2048], F16, tag="wkv", bufs=1, name="wkv_sb")
    if not loop:
        ld(nc.scalar, wkv_sb[:], tens["wkv"][:])
    wk_t = [wkv_sb[:, 128 * k:128 * (k + 1)] for k in range(KT)]
    wv_t = [wkv_sb[:, 2048 + 128 * k:2048 + 128 * (k + 1)]
            for k in range(KT)]
    ck_sb = wkv_sb[:, 4096:6144]
    sk_sb = wkv_sb[:, 6144:8192]
    wq_big = sb.tile([128, KT * 512], F16, tag="wq", bufs=1, name="wq_big")
    ld(nc.scalar, wq_big[:, 0:4 * 512], tens["wq"][:, 0:4, :])
    wq_t = [wq_big[:, 512 * k:512 * (k + 1)] for k in range(KT)]
    # q-side rope tables derived on ACT (cq = ck*lam): saves 1MB of input
    # DMA + two triggers per iteration; emitted after the wqA trigger so
    # the 2x ~2us ACT ops don't delay wq's descriptors
    LAM = HD ** -0.5
    COPY = mybir.ActivationFunctionType.Copy
    cq_sb = sb.tile([128, T], F16, tag="cq", name="cq_sb")
    nc.scalar.activation(cq_sb[:], ck_sb, COPY, scale=LAM)
    sq_sb = sb.tile([128, T], F16, tag="sq", name="sq_sb")
    nc.scalar.activation(sq_sb[:], sk_sb, COPY, scale=LAM)

    # gpsimd ring: ident, mask, xb2; onesf + bias via memset
    id_sb = sb.tile([128, 128], F16, tag="ident", name="id_sb")
    ld(nc.gpsimd, id_sb[:], tens["ident"][:])
    mask_sb = sb.tile([128, 2048], F16, tag="mask", name="mask_sb")
    ld(nc.gpsimd, mask_sb[:], tens["masks"][:])
    ld(nc.gpsimd, xb[2][:], tens["xT"][:, 2, :])
    onesf_sb = sb.tile([128, 128], F16, tag="onesf", name="onesf_sb")
    nc.gpsimd.memset(onesf_sb[:], 1.0)
    bias_sb = sb.tile([128, 1], F32, tag="bias", name="bias_sb")
    nc.gpsimd.memset(bias_sb[:], EXP_BIAS)
    # dummy exp: hoists ACT's exp-table load off the critical path
    wex = sb.tile([128, 1], F16, tag="wex", name="wex")
    nc.scalar.activation(wex[:], wkv_sb[:, 0:1], EXP)

    # sync ring: xb1 (behind prev iteration's trailing y stores only)
    ld(nc.sync, xb[1][:], tens["xT"][:, 1, :])
    if not loop:
        # graded single-shot build: xb0 up front (no cross-iter prefetch)
        ld(nc.sync, xb[0][:], tens["xT"][:, 0, :])

    def tiny_out(src_ap):
        osb0 = sb.tile([128, 512], F16, tag="osb", bufs=2, name="osb0")
        nc.vector.tensor_copy(osb0[:], src_ap)
        nc.sync.dma_start(tens["y"][0:128, :], osb0[:])

    if phase == "noop":
        if loop:
            ld(nc.scalar, wkv_sb[:], tens["wkv"][:])
        ld(nc.scalar, xb[3][:], tens["xT"][:, 3, :])
        ld(nc.scalar, wq_big[:, 4 * 512:], tens["wq"][:, 4:16, :])
        wo_bigN = sb.tile([128, KT * 512], F16, tag="wo", bufs=1,
                          name="wo_bigN")
        ld(nc.scalar, wo_bigN[:], tens["wo"][:, :, :])
        if loop:
            ld(nc.gpsimd, xb[0][:], tens["xT"][:, 0, :])
        tiny_out(xb[1][:, 0:512])
        return

    kT_sb = sb.tile([128, T], F16, tag="kT", name="kT_sb")
    vT_sb = sb.tile([128, T], F16, tag="vT", name="vT_sb")
    vn = [sb.tile([128, 512], F16, tag="vn", bufs=4, name=f"vn{q}")
          for q in range(4)]
    qT = [sb.tile([128, T], F16, tag="qT", bufs=2, name=f"qT{h}")
          for h in range(GQ)]
    aoT = [sb.tile([128, T], AOD, tag="aoT", bufs=2, name=f"aoT{h}")
           for h in range(GQ)]

    # ---- phase A: per 512-token block: k/v proj + rope + v transpose
    # + q-proj h0.  ~22us PE per 2MB block: chase-proof and HAM-warm.
    wo_t = None
    for n in range(NT):
        nsl = slice(512 * n, 512 * (n + 1))
        pk = ps.tile([128, 512], F32, tag="proj", bufs=PROJ_B, name=f"pk{n}")
        for k in range(KT):
            nc.tensor.matmul(pk[:], wk_t[k][:], xcol[n][k],
                             start=(k == 0), stop=(k == KT - 1))
        kraw = sb.tile([128, 512], F16, tag="qraw", bufs=2, name=f"kraw{n}")
        nc.scalar.copy(kraw[:], pk[:])
        _rope(nc, sb, kT_sb, kraw, ck_sb, sk_sb, nsl,
              heavy=nc.vector)

        pv = ps.tile([128, 512], F32, tag="proj", bufs=PROJ_B, name=f"pv{n}")
        for k in range(KT):
            nc.tensor.matmul(pv[:], wv_t[k][:], xcol[n][k],
                             start=(k == 0), stop=(k == KT - 1))
        nc.scalar.copy(vT_sb[:, nsl], pv[:])

        pt = ps.tile([128, 512], F16, tag="proj", bufs=PROJ_B, name=f"pt{n}")
        for i in range(4):
            j = 4 * n + i
            nc.tensor.matmul(pt[:, 128 * i:128 * (i + 1)],
                             vT_sb[:, 128 * j:128 * (j + 1)], id_sb[:],
                             is_transpose=True,
                             start=(i == 0), stop=(i == 3))
        nc.scalar.copy(vn[n][:], pt[:])

        if n == 0:
            # rest of wq must precede this first q-proj's k>=4 matmuls
            ld(nc.scalar, wq_big[:, 4 * 512:], tens["wq"][:, 4:16, :])
        pj = ps.tile([128, 512], F32, tag="proj", bufs=PROJ_B, name=f"pq0_{n}")
        for k in range(KT):
            nc.tensor.matmul(pj[:], wq_t[k][:, 0:128], xcol[n][k],
                             start=(k == 0), stop=(k == KT - 1))
        qraw = sb.tile([128, 512], F16, tag="qraw", bufs=2,
                       name=f"qraw0_{n}")
        nc.vector.tensor_copy(qraw[:], pj[:])
        _rope(nc, sb, qT[0], qraw, cq_sb, sq_sb, nsl,
              heavy=nc.vector)

        if n == 0:
            # late trigger on the scalar ring: xb3 (read only at n=3)
            ld(nc.scalar, xb[3][:], tens["xT"][:, 3, :])
        if n == 1:
            wo_big = sb.tile([128, KT * 512], F16, tag="wo", bufs=1,
                             name="wo_big")
            ld(nc.scalar, wo_big[:], tens["wo"][:, :, :])
            wo_t = [wo_big[:, 512 * k:512 * (k + 1)] for k in range(KT)]

    def qproj(h):
        hsl = slice(128 * h, 128 * (h + 1))
        for n in range(NT):
            nsl = slice(512 * n, 512 * (n + 1))
            pj = ps.tile([128, 512], F32, tag="proj", bufs=PROJ_B,
                         name=f"pjq{h}_{n}")
            for k in range(KT):
                nc.tensor.matmul(pj[:], wq_t[k][:, hsl], xcol[n][k],
                                 start=(k == 0), stop=(k == KT - 1))
                yield True
            qraw = sb.tile([128, 512], F16, tag="qraw", bufs=2,
                           name=f"qraw{h}_{n}")
            nc.vector.tensor_copy(qraw[:], pj[:])
            _rope(nc, sb, qT[h], qraw, cq_sb, sq_sb, nsl,
                  heavy=nc.vector if ar_dve else None)
            if loop and h == 2 and n == 3:
                # next iteration's weight/table blob; this iteration's
                # readers (k/v-proj, ropes) finished during attention 0/1
                ld(nc.scalar, wkv_sb[:], tens["wkv"][:])
            if loop and h == 3 and n == 3:
                # next iteration's first block; readers of xb0 (this
                # iteration's projections) are all done by here
                ld(nc.gpsimd, xb[0][:], tens["xT"][:, 0, :])

    if phase == "proj":
        for h in range(1, 4):
            for _ in qproj(h):
                pass
        tiny_out(qT[3][:, 0:512])
        return

    # ---- attention per head, interleaved with next head's q-proj ----
    LOOK = look

    def attention(h, filler, fill_per_step=2, fill_start=0):
        blocks = [(n, j) for n in range(NT) for j in range(4 * (n + 1))]
        nsteps = len(blocks) + LOOK
        outT = {}
        dBs = {}
        e_acc = {}
        eTs = {}
        for step in range(nsteps):
            if step < len(blocks):
                n, j = blocks[step]
                nsl = slice(512 * n, 512 * (n + 1))
                if j == 0:
                    outT[n] = ps.tile([128, 512], F32, tag="outT", bufs=2,
                                      name=f"outT{h}_{n}")
                    dBs[n] = ps.tile([128, 512], F32, tag="outT", bufs=2,
                                     name=f"dB{h}_{n}")
                    e_acc[n] = sb.tile([128, 512], F16, tag="eacc", bufs=2,
                                       name=f"eacc{h}_{n}")
                r = j - 4 * n          # >= 0 only for diagonal blocks
                q0 = 128 * r if r > 0 else 0
                w = 512 - q0
                sT = ps.tile([128, 512], F32, tag="sblk", bufs=SBLK_B,
                             name=f"sT{h}_{n}_{j}")
                nc.tensor.matmul(sT[:, 0:w],
                                 kT_sb[:, 128 * j:128 * (j + 1)],
                                 qT[h][:, 512 * n + q0:512 * (n + 1)],
                                 start=True, stop=True)
                eT = sb.tile([128, 512], F16, tag="eT", bufs=LOOK + 2,
                             name=f"eT{h}_{n}_{j}")
                nc.scalar.activation(eT[:, 0:w], sT[:, 0:w], EXP,
                                     bias=bias_sb[:])
                if r >= 0:  # diagonal block: causal mask on the valid range
                    nc.vector.tensor_mul(
                        eT[:, 0:w], eT[:, 0:w],
                        mask_sb[:, 512 * r + q0:512 * (r + 1)])
                if j == 0:
                    nc.vector.tensor_copy(e_acc[n][:], eT[:])
                else:
                    nc.vector.tensor_add(e_acc[n][:, q0:512],
                                         e_acc[n][:, q0:512], eT[:, 0:w])
                eTs[(n, j)] = (eT, q0, w)
            if step >= LOOK:
                n, j = blocks[step - LOOK]
                nsl = slice(512 * n, 512 * (n + 1))
                eT_j, q0, w = eTs.pop((n, j))
                nc.tensor.matmul(outT[n][:, q0:512],
                                 vn[j // 4][:, 128 * (j % 4):128 * (j % 4 + 1)],
                                 eT_j[:, 0:w], start=(j == 0),
                                 stop=(j == 4 * (n + 1) - 1))
                if j == 4 * (n + 1) - 1:
                    dB = dBs[n]
                    nc.tensor.matmul(dB[:], onesf_sb[:], e_acc[n][:],
                                     start=True, stop=True)
                    rD = sb.tile([128, 512], F32, tag="rD", bufs=2,
                                 name=f"rD{h}_{n}")
                    nc.vector.reciprocal_approx_fast(out=rD[:], in_=dB[:])
                    nc.vector.tensor_mul(aoT[h][:, nsl], outT[n][:], rD[:])
            if filler is not None and step >= fill_start:
                for _ in range(fill_per_step):
                    if next(filler, None) is None:
                        filler = None
                        break
        return filler

    # DRAM staging + collectives (3D so reloads can slice 128-row blocks)
    ao_in01 = dram.tile([2, 128, T], AOD, tag="ao_in01", bufs=2,
                        name="ao_in01")
    ao_c01 = dram.tile([8, 128, T], AOD, tag="ao_c01", bufs=2, name="ao_c01")
    ao_in2 = dram.tile([1, 128, T], AOD, tag="ao_in2", bufs=2, name="ao_in2")
    ao_c2 = dram.tile([4, 128, T], AOD, tag="ao_c2", bufs=2, name="ao_c2")
    ao_in3 = dram.tile([1, 128, T], AOD, tag="ao_in3", bufs=2, name="ao_in3")
    ao_c3 = dram.tile([4, 128, T], AOD, tag="ao_c3", bufs=2, name="ao_c3")

    def all_gather(ao_in, ao_c, nrows):
        if fake_ag:
            nb = ao_in.shape[0]
            engs = [nc.gpsimd, nc.sync, nc.scalar]
            for gg in range(1, 4):
                engs[gg - 1].dma_start(ao_c[nb * gg:nb * (gg + 1), :, :],
                                       ao_in[:, :, :])
        else:
            nc.gpsimd.collective_compute(
                "AllGather", mybir.AluOpType.bypass,
                replica_groups=[[0, 1, 2, 3], [4, 5, 6, 7]],
                ins=[ao_in.opt()], outs=[ao_c.opt()],
            )

    filler = qproj(1)
    filler = attention(0, filler)
    if filler is not None:
        for _ in filler:
            pass
    nc.sync.dma_start(ao_in01[0, :, :], aoT[0][:])

    filler = qproj(2)
    filler = attention(1, filler)
    if filler is not None:
        for _ in filler:
            pass
    nc.sync.dma_start(ao_in01[1, :, :], aoT[1][:])
    all_gather(ao_in01, ao_c01, 256)

    filler = qproj(3)
    filler = attention(2, filler)
    if filler is not None:
        for _ in filler:
            pass
    # Reload gathered tiles into buffers that died early (wq after qproj3,
    # cq/sq after the last q rope, vT after the v transposes, qT ring slot A
    # after attention h2).  NOT xb: xb slots must free for next-iter reloads.
    ao_wq = sb.tile([128, 4 * T], AOD, tag="wq", bufs=1, name="ao_wq")
    ao_cq = sb.tile([128, T], AOD, tag="cq", bufs=1, name="ao_cq")
    ao_sq = sb.tile([128, T], AOD, tag="sq", bufs=1, name="ao_sq")
    ao_vT = sb.tile([128, T], AOD, tag="vT", bufs=1, name="ao_vT")
    ao_qTa = sb.tile([128, T], AOD, tag="qT", bufs=2, name="ao_qTa")
    aoA = [ao_wq[:, T * i:T * (i + 1)] for i in range(4)] + \
          [ao_cq[:], ao_sq[:], ao_vT[:], ao_qTa[:]]
    engs3 = [nc.scalar, nc.sync, nc.gpsimd]
    for r in range(8):
        engs3[r % 3].dma_start(aoA[r], ao_c01[r, :, :])
    nc.sync.dma_start(ao_in2[0, :, :], aoT[2][:])
    all_gather(ao_in2, ao_c2, 128)

    oA = []

    def passA():
        for m in range(KT):
            po = ps.tile([128, 512], F32, tag="proj", bufs=PROJ_B,
                         name=f"poA{m}")
            for i in range(8):
                nc.tensor.matmul(po[:], aoA[i][:, 128 * m:128 * (m + 1)],
                                 wo_t[MAP01[i]][:],
                                 start=(i == 0), stop=(i == 7))
                yield True
            t = sb.tile([128, 512], F16, tag="oA", bufs=KT, name=f"oA{m}")
            eng = nc.vector if pa_fill else nc.scalar
            eng.tensor_copy(t[:], po[:]) if pa_fill else nc.scalar.copy(
                t[:], po[:])
            oA.append(t)

    pa = attention(3, passA() if pa_fill else None, fill_per_step=3)
    ao_kT = sb.tile([128, T], AOD, tag="kT", bufs=1, name="ao_kT")
    ao_mk = sb.tile([128, T], AOD, tag="mask", bufs=1, name="ao_mk")
    ao_aTa = sb.tile([128, T], AOD, tag="aoT", bufs=2, name="ao_aTa")
    ao_qTb = sb.tile([128, T], AOD, tag="qT", bufs=2, name="ao_qTb")
    aoC = [ao_aTa[:], ao_qTb[:], ao_kT[:], ao_mk[:]]
    for r in range(4):
        engs3[(r + 2) % 3].dma_start(aoC[r], ao_c2[r, :, :])
    if pa_fill and pa is not None:
        for _ in pa:
            pass
    if phase == "attn":
        tiny_out(aoT[3][:, 0:512])
        return
    if not pa_fill:
        for _ in passA():
            pass
    nc.sync.dma_start(ao_in3[0, :, :], aoT[3][:])
    all_gather(ao_in3, ao_c3, 128)
    ao_aTb = sb.tile([128, T], AOD, tag="aoT", bufs=2, name="ao_aTb")
    ao_ex = sb.tile([128, 3 * T], AOD, tag="ao_ex", bufs=1, name="ao_ex")
    aoD = [ao_aTb[:]] + [ao_ex[:, T * i:T * (i + 1)] for i in range(3)]
    for r in range(4):
        engs3[r % 3].dma_start(aoD[r], ao_c3[r, :, :])

    for m in range(KT):
        po = ps.tile([128, 512], F32, tag="proj", bufs=PROJ_B, name=f"poCD{m}")
        for i in range(4):
            nc.tensor.matmul(po[:], aoC[i][:, 128 * m:128 * (m + 1)],
                             wo_t[MAP2[i]][:], start=(i == 0), stop=False)
        for i in range(4):
            nc.tensor.matmul(po[:], aoD[i][:, 128 * m:128 * (m + 1)],
                             wo_t[MAP3[i]][:], start=False, stop=(i == 3))
        osb = sb.tile([128, 512], F16, tag="osb", bufs=2, name=f"osb{m}")
        nc.vector.tensor_add(osb[:], po[:], oA[m][:])
        nc.sync.dma_start(tens["y"][128 * m:128 * (m + 1), :], osb[:])


def _rope(nc, sb, dst, raw, c2, s2, nsl, heavy=None):
    """dst[:, nsl] = rotate(raw); rows 0:64 real, 64:128 imag.

    Two full-width muls on DVE exploit the duplicated-half cos/sin
    layout (c2 = [c; c], s2 = [s; s]):  mA = raw*c2 = [r*c; i*c],
    mB = raw*s2 = [r*s; i*s]; then real = mA[hi]-mB[lo], imag =
    mB[hi]+mA[lo].  add/sub run on `heavy` (Pool during attention
    phases where DVE carries the softmax chain, DVE during proj)."""
    heavy = heavy or nc.gpsimd
    mA = sb.tile([128, 512], F16, tag="rs", bufs=4, name="mA")
    mBs = sb.tile([128, 512], F16, tag="rs", bufs=4, name="mBs")
    nc.vector.tensor_mul(mA[:], raw[:], c2[:, nsl])
    # mBs holds [i*s; r*s]: half-swapped so the combine ops' SBUF inputs
    # share a base partition (walrus checkSBSameStartPartition)
    nc.vector.tensor_mul(mBs[0:64, :], raw[64:128, :], s2[64:128, nsl])
    nc.vector.tensor_mul(mBs[64:128, :], raw[0:64, :], s2[0:64, nsl])
    heavy.tensor_sub(dst[0:64, nsl], mA[0:64, :], mBs[0:64, :])
    heavy.tensor_add(dst[64:128, nsl], mBs[64:128, :], mA[64:128, :])


# ---------------------------------------------------------------------
_NC_CACHE = {}


def _get_nc():
    if "nc" not in _NC_CACHE:
        _NC_CACHE["nc"] = build_nc()
    return _NC_CACHE["nc"]


def _deinterleave(w):
    # per head: col order [0,2,4,...,126, 1,3,...,127]
    d, c = w.shape
    nh = c // HD
    wh = w.reshape(d, nh, HD // 2, 2)
    return np.concatenate([wh[..., 0], wh[..., 1]], axis=-1).reshape(d, c)


def make_inputs(x, freqs_cos, freqs_sin, wq, wk, wv, wo, ao8=True):
    x = np.asarray(x, dtype=np.float32)
    xT = [np.ascontiguousarray(x[b].T.astype(np.float16)) for b in range(B)]
    cosT = np.asarray(freqs_cos, dtype=np.float64).T  # [64, T]
    sinT = np.asarray(freqs_sin, dtype=np.float64).T
    lam = HD ** -0.5
    ck_np = np.concatenate([cosT, cosT], axis=0).astype(np.float16)
    sk_np = np.concatenate([sinT, sinT], axis=0).astype(np.float16)
    wq_p = _deinterleave(np.asarray(wq, dtype=np.float32)).astype(np.float16)
    wk_p = _deinterleave(np.asarray(wk, dtype=np.float32)).astype(np.float16)
    vsc = 4.0 if ao8 else 1.0
    wv16 = (np.asarray(wv, dtype=np.float32) * vsc).astype(np.float16)
    wo16 = (np.asarray(wo, dtype=np.float32) / vsc).astype(np.float16)

    mask = np.zeros((128, 2048), dtype=np.float16)
    ii = np.arange(128)[:, None]
    cc = np.arange(512)[None, :]
    for r in range(4):
        mask[:, 512 * r:512 * (r + 1)] = (cc >= 128 * r + ii)
    ident = np.eye(128, dtype=np.float16)

    def shuf(w):
        # [2048, C] -> [128, 16, C]: element [p, k, c] = w[128k + p, c]
        d, c = w.shape
        return np.ascontiguousarray(
            w.reshape(KT, 128, c).transpose(1, 0, 2))

    def shuf_x(xb):
        # [2048(dim), 2048(tok)] -> [128, NT, KT*512]:
        # [p, n, 512k + c] = xb[128k + p, 512n + c]
        a = xb.reshape(KT, 128, NT, 512)        # [k, p, n, c]
        return np.ascontiguousarray(
            a.transpose(1, 2, 0, 3)).reshape(128, NT, KT * 512)

    in_maps = []
    for core in range(8):
        b, g = core // 4, core % 4
        wkv = np.concatenate([
            shuf(wk_p[:, 128 * g:128 * (g + 1)]).reshape(128, 2048),
            shuf(wv16[:, 128 * g:128 * (g + 1)]).reshape(128, 2048),
            ck_np, sk_np], axis=1)
        in_maps.append({
            "xT": shuf_x(xT[b]),
            "wq": shuf(wq_p[:, 512 * g:512 * (g + 1)]),
            "wkv": np.ascontiguousarray(wkv),
            "wo": shuf(wo16[:, 512 * g:512 * (g + 1)]),
            "masks": mask, "ident": ident,
        })
    return in_maps


def kernel(x, freqs_cos, freqs_sin, wq, wk, wv, wo):
    nc = _get_nc()
    in_maps = make_inputs(x, freqs_cos, freqs_sin, wq, wk, wv, wo)
    res = run_bass_kernel_spmd(nc, in_maps, core_ids=list(range(8)))
    out = np.empty((B, T, DIM), dtype=np.float32)
    for core in range(8):
        b, g = core // 4, core % 4
        out[b][:, 512 * g:512 * (g + 1)] = \
            res.results[core]["y"].astype(np.float32)
    return out
